# revision 8
# baseline (speedup 1.0000x reference)
"""Keras-LSTM layer kernel for 8 Trainium2 NeuronCores.

Sharding: data-parallel over batch (B=64 -> 8 per core); kernel/recurrent
weights and bias replicated. Each core computes the input projection
x_proj = x @ Wx + bias for its batch slice (big efficient matmul, 128-row
M-tiles), then runs the sequential 512-step LSTM scan locally:
    z_t = x_proj_t + h_{t-1} @ Wh   (PSUM, 4-way column-tiled: one 32-row
                                     strip per gate i/f/g/o)
    i,f,o = sigmoid(..); g = tanh(..); c = f*c + i*g; h = o*tanh(c)
No cross-core communication (remote DMA / collectives are not usable per
step on this runtime), so the scan is fully local per batch shard.
"""

import sys

sys.path.insert(0, "/opt/trn_rl_repo")

import numpy as np

import concourse.bass as bass
import concourse.mybir as mybir
import concourse.tile as tile
from concourse import bacc
from concourse.bass import ds
from concourse.bass_utils import run_bass_kernel_spmd
from concourse.masks import make_identity

B, T, D, U = 64, 512, 1024, 1024
G = 4 * U
NCORES = 8
BPC = B // NCORES  # batch rows per core
ROWS = T * BPC  # 4096 (t-major row index = t*BPC + b)
F32 = mybir.dt.float32

_CACHE = {}


def _build(unroll=2):
    nc = bacc.Bacc("TRN2", target_bir_lowering=False, debug=False,
                   num_devices=NCORES)
    x = nc.dram_tensor("x", [D, ROWS], F32, kind="ExternalInput").ap()
    wx = nc.dram_tensor("wx", [D, G], F32, kind="ExternalInput").ap()
    wh = nc.dram_tensor("wh", [D, G], F32, kind="ExternalInput").ap()
    bias = nc.dram_tensor("bias", [1, G], F32, kind="ExternalInput").ap()
    y = nc.dram_tensor("y", [ROWS, U], F32, kind="ExternalOutput").ap()
    xproj = nc.dram_tensor("xproj", [ROWS, G], F32).ap()

    with tile.TileContext(nc, trace_sim=False) as tc:
        with tc.tile_pool(name="const", bufs=1) as cpool:
            ones = cpool.tile([1, 128], F32)
            nc.gpsimd.memset(ones[:], 1.0)
            i8 = cpool.tile([8, 8], F32)
            make_identity(nc, i8[:])

            # ---------------- phase 1: xproj = x @ Wx + bias ----------------
            with tc.tile_pool(name="wxp", bufs=1) as wxp, \
                 tc.tile_pool(name="p1sb", bufs=3) as p1sb, \
                 tc.tile_pool(name="p1xt", bufs=2) as p1xt, \
                 tc.tile_pool(name="p1ps", bufs=2, space="PSUM") as p1ps:
                bias_sb = wxp.tile([1, G], F32)
                nc.sync.dma_start(bias_sb[:], bias[:])
                wx_sb = wxp.tile([128, 8 * G], F32)
                for k in range(8):
                    nc.sync.dma_start(wx_sb[:, k * G:(k + 1) * G],
                                      wx[k * 128:(k + 1) * 128, :])
                with tc.For_i(0, ROWS, 128) as m:
                    xt = p1xt.tile([128, 1024], F32, tag="xt")
                    for k in range(8):
                        nc.sync.dma_start(
                            xt[:, k * 128:(k + 1) * 128],
                            x[k * 128:(k + 1) * 128, ds(m, 128)])
                    for n in range(8):
                        p1 = p1ps.tile([128, 512], F32, tag="p1")
                        nc.tensor.matmul(p1[:], ones[:],
                                         bias_sb[:, n * 512:(n + 1) * 512],
                                         start=True, stop=False)
                        for k in range(8):
                            nc.tensor.matmul(
                                p1[:], xt[:, k * 128:(k + 1) * 128],
                                wx_sb[:, k * G + n * 512:k * G + (n + 1) * 512],
                                start=False, stop=(k == 7))
                        xp_sb = p1sb.tile([128, 512], F32, tag="xp")
                        nc.scalar.copy(xp_sb[:], p1[:])
                        nc.sync.dma_start(
                            xproj[ds(m, 128), n * 512:(n + 1) * 512], xp_sb[:])

            # ---------------- phase 2: sequential LSTM scan -----------------
            with tc.tile_pool(name="whp", bufs=1) as whp, \
                 tc.tile_pool(name="state", bufs=1) as st, \
                 tc.tile_pool(name="gate", bufs=1) as gp, \
                 tc.tile_pool(name="xpt", bufs=2) as xptp, \
                 tc.tile_pool(name="p2ps", bufs=2, space="PSUM") as p2ps, \
                 tc.tile_pool(name="p2pt", bufs=2, space="PSUM") as p2pt:
                wh_sb = whp.tile([128, 8 * G], F32)
                for k in range(8):
                    nc.sync.dma_start(wh_sb[:, k * G:(k + 1) * G],
                                      wh[k * 128:(k + 1) * 128, :])
                c_t = st.tile([8, U], F32)
                hT = st.tile([128, 64], F32)
                nc.gpsimd.memset(c_t[:], 0.0)
                nc.gpsimd.memset(hT[:], 0.0)

                def step(row):
                    # row = dynamic DRAM row offset (t*BPC)
                    xp_t = xptp.tile([8, G], F32, tag="xp_t")
                    nc.sync.dma_start(xp_t[:], xproj[ds(row, 8), :])
                    zt = p2ps.tile([128, 1024], F32, tag="zt")
                    # inject x_proj_t into PSUM strips (start=True) then
                    # accumulate h @ Wh on top. strip c <-> gate block c.
                    for c in range(4):
                        sp = zt[32 * c:32 * c + 8, :]
                        for h2 in range(2):
                            nc.tensor.matmul(
                                sp[:, h2 * 512:(h2 + 1) * 512], i8[:],
                                xp_t[:, c * 1024 + h2 * 512:
                                     c * 1024 + (h2 + 1) * 512],
                                start=True, stop=False,
                                tile_position=(0, 32 * c))
                    for k in range(8):
                        for c in range(4):
                            sp = zt[32 * c:32 * c + 8, :]
                            for h2 in range(2):
                                nc.tensor.matmul(
                                    sp[:, h2 * 512:(h2 + 1) * 512],
                                    hT[:, 8 * k:8 * k + 8],
                                    wh_sb[:, k * G + c * 1024 + h2 * 512:
                                          k * G + c * 1024 + (h2 + 1) * 512],
                                    start=False, stop=(k == 7),
                                    tile_position=(0, 32 * c))
                    sig_i = gp.tile([8, U], F32, tag="si")
                    sig_f = gp.tile([8, U], F32, tag="sf")
                    tg = gp.tile([8, U], F32, tag="tg")
                    sig_o = gp.tile([8, U], F32, tag="so")
                    Sig = mybir.ActivationFunctionType.Sigmoid
                    Tanh = mybir.ActivationFunctionType.Tanh
                    nc.scalar.activation(sig_f[:], zt[32:40, :], Sig)
                    nc.scalar.activation(sig_i[:], zt[0:8, :], Sig)
                    nc.scalar.activation(tg[:], zt[64:72, :], Tanh)
                    nc.scalar.activation(sig_o[:], zt[96:104, :], Sig)
                    itg = gp.tile([8, U], F32, tag="itg")
                    fc = gp.tile([8, U], F32, tag="fc")
                    nc.vector.tensor_mul(fc[:], sig_f[:], c_t[:])
                    nc.vector.tensor_mul(itg[:], sig_i[:], tg[:])
                    nc.vector.tensor_add(c_t[:], fc[:], itg[:])
                    tc_t = gp.tile([8, U], F32, tag="tg")
                    nc.scalar.activation(tc_t[:], c_t[:], Tanh)
                    h = gp.tile([8, U], F32, tag="si")
                    nc.vector.tensor_mul(h[:], sig_o[:], tc_t[:])
                    # transpose h -> hT chunks for next step's stationary
                    hT_ps = p2pt.tile([128, 64], F32, tag="htp")
                    for k in range(8):
                        nc.tensor.transpose(hT_ps[:, 8 * k:8 * k + 8],
                                            h[:, 128 * k:128 * (k + 1)],
                                            i8[:])
                    nc.vector.tensor_copy(hT[:], hT_ps[:])
                    nc.sync.dma_start(y[ds(row, 8), :], h[:])

                with tc.For_i(0, ROWS, 8 * unroll) as r:
                    for s in range(unroll):
                        step(r + 8 * s)

    nc.compile()
    return nc


def _get_nc():
    if "nc" not in _CACHE:
        _CACHE["nc"] = _build()
    return _CACHE["nc"]


def kernel(inputs, kernel, recurrent_kernel, bias):
    nc = _get_nc()
    in_maps = []
    for j in range(NCORES):
        xj = np.ascontiguousarray(
            inputs[j * BPC:(j + 1) * BPC].transpose(1, 0, 2).reshape(ROWS, D).T)
        in_maps.append({
            "x": np.asarray(xj, np.float32),
            "wx": np.asarray(kernel, np.float32),
            "wh": np.asarray(recurrent_kernel, np.float32),
            "bias": np.asarray(bias, np.float32).reshape(1, G),
        })
    res = run_bass_kernel_spmd(nc, in_maps, list(range(NCORES)))
    outs = []
    for j in range(NCORES):
        yj = res.results[j]["y"].reshape(T, BPC, U).transpose(1, 0, 2)
        outs.append(yj)
    return np.ascontiguousarray(np.concatenate(outs, axis=0), dtype=np.float32)
